# revision 29
# baseline (speedup 1.0000x reference)
"""Trainium2 Bass kernel v3 for nn_MultiHeadAttention_81655918232272.

Reference semantics:
    q = (x @ Wq).reshape(B, N, H, Dh)   # H=16 heads, Dh=64 (biases zero)
    scores = einsum("bnhd,bngd->bnhg", q, k)   # per-token 16x16 head-mixing
    ctx = softmax(scores, -1) @ v ; out = ctx.reshape(.., 1024) @ Wo

Design v3 (per core: 4096 tokens = 8 macrotiles x 512 tokens):
  - q,k computed TRANSPOSED via swapped-operand matmuls (lhsT=W chunk,
    rhs=xT chunk, N=512): psum [128=(2 heads x 64d), 512 tok]; DVE
    copies build Gq/Gk [64 d, (t512 h16)-interleaved] straight from
    psum -> no PE transposes for q/k.
  - v computed forward per 128-token subtile; ones-column fused vt via
    coarse DMA shuffle through DRAM staging (as v2).
  - scores: per 4-grp chunk, 4 matmuls K=64 -> [128,512] psum; exp on
    ACT; block-diag mask multiply on DVE -> E chunk.
  - ctx: 4 matmuls N=65 -> [128,260] psum -> ctxu; per-subtile DMA
    shuffle back to [tok, (h,65)]; normalize; PE transpose; Wo forward.
  - software pipeline: attention of macrotile m-1 interleaved between
    the projection chunks of macrotile m; ctx staggered 2 chunks behind
    scores so the exp/mask chain never stalls the PE.
"""

import numpy as np

H = 16
DH = 64
DIM = 1024
B, N = 32, 1024
NCORES = 8
BPC = B // NCORES          # batches per core
T = BPC * N                # tokens per core (4096)
NTILE = T // 128           # 128-token tiles per core (32)
TM = 512                   # macrotile tokens
NMACRO = T // TM           # 8

_CACHE = {}


def _build(nmacro=NMACRO, debug=False):
    import concourse.bass as bass  # noqa: F401
    import concourse.mybir as mybir
    import concourse.tile as tile
    from concourse import bacc
    from concourse.masks import make_identity
    from contextlib import ExitStack

    bf16, fp32 = mybir.dt.bfloat16, mybir.dt.float32
    fp16 = mybir.dt.float16
    Exp = mybir.ActivationFunctionType.Exp
    Tl = TM * nmacro

    nc = bacc.Bacc(None, target_bir_lowering=False, debug=debug)

    with tile.TileContext(nc) as tc, ExitStack() as ctx:
        dram = ctx.enter_context(tc.tile_pool(name="dram", bufs=1, space="DRAM"))
        const = ctx.enter_context(tc.tile_pool(name="const", bufs=1))
        sbA = ctx.enter_context(tc.tile_pool(name="sbA", bufs=2))
        sbB = ctx.enter_context(tc.tile_pool(name="sbB", bufs=2))
        dstage = ctx.enter_context(tc.tile_pool(name="dstage", bufs=2, space="DRAM"))
        proj_ps = ctx.enter_context(tc.tile_pool(name="proj_ps", bufs=2, space="PSUM"))
        sc_ps = ctx.enter_context(tc.tile_pool(name="sc_ps", bufs=2, space="PSUM"))
        cx_ps = ctx.enter_context(tc.tile_pool(name="cx_ps", bufs=2, space="PSUM"))
        tr_ps = ctx.enter_context(tc.tile_pool(name="tr_ps", bufs=2, space="PSUM"))

        # ---- DRAM I/O ----
        xT_d = dram.tile([DIM, Tl], fp16, kind="ExternalInput")
        w_d = {}
        for wname in ("wq", "wk", "wv", "wo"):
            w_d[wname] = dram.tile([DIM, DIM], fp16, kind="ExternalInput",
                                   name=f"{wname}_d")
        mask_d = dram.tile([128, 512], bf16, kind="ExternalInput")
        out_d = dram.tile([Tl, DIM], fp32, kind="ExternalOutput")

        # ---- resident SBUF ----
        w_sb = {}
        for wname in ("wq", "wk", "wv", "wo"):
            wt = const.tile([128, 8 * DIM], fp16, tag=f"w_{wname}", name=f"w_{wname}_sb")
            for kt in range(8):
                nc.sync.dma_start(wt[:, DIM * kt:DIM * (kt + 1)],
                                  w_d[wname][128 * kt:128 * (kt + 1), :])
            w_sb[wname] = wt
        mask_sb = const.tile([128, 512], bf16)
        nc.sync.dma_start(mask_sb[:], mask_d[:])
        ident = const.tile([128, 128], fp16)
        make_identity(nc, ident[:])

        def load_xt(m):
            t0 = TM * m
            xt = sbA.tile([128, 8, TM], fp16, tag="xt", name="xt", bufs=2)
            nc.sync.dma_start(
                xt[:],
                xT_d[:, t0:t0 + TM].rearrange("(kt f) t -> f kt t", f=128))
            return xt

        def proj_qkT(j, xt, G, wname):
            """Transposed projection chunk j: psum [(2h,64d), 512 tok] ->
            DVE-interleave into G [64, (t 512, h 16)]."""
            ps = proj_ps.tile([128, TM], fp32, tag="proj", name="proj_ps")
            for kt in range(8):
                nc.tensor.matmul(
                    ps[:],
                    w_sb[wname][:, DIM * kt + 128 * j:DIM * kt + 128 * (j + 1)],
                    xt[:, kt, :],
                    start=(kt == 0), stop=(kt == 7))
            # dense ACT cast psum->fp16, then DVE interleave into G.
            # G col order: grp*128 + h*8 + t_l  (h-major within group)
            qs = sbA.tile([128, TM], fp16, tag="qks", name="qT_sb", bufs=2)
            nc.scalar.copy(qs[:], ps[:])
            Gv = G[:].rearrange("d (grp hh t) -> d grp hh t", hh=H, t=8)
            for p in range(2):
                nc.vector.tensor_copy(
                    Gv[:, :, 2 * j + p, :],
                    qs[64 * p:64 * (p + 1), :].rearrange("d (grp t) -> d grp t", t=8))

        def proj_v(s, xt, vts):
            """Forward V projection for subtile s + vt shuffle."""
            v16 = sbA.tile([128, DIM], bf16, tag="v16", name="v16", bufs=2)
            pss = [proj_ps.tile([128, TM], fp32, tag="proj", name="proj_ps")
                   for _ in range(2)]
            for n in range(2):
                for kt in range(8):
                    nc.tensor.matmul(
                        pss[n][:, 0:512],
                        xt[:, kt, 128 * s:128 * (s + 1)],
                        w_sb["wv"][:, DIM * kt + 512 * n:DIM * kt + 512 * (n + 1)],
                        start=(kt == 0), stop=(kt == 7))
                nc.scalar.copy(v16[:, 512 * n:512 * (n + 1)], pss[n][:, 0:512])

            v_dr = dstage.tile([128, DIM], bf16, tag="v_dr", name="v_dr", bufs=2)
            nc.sync.dma_start(v_dr[:], v16[:])
            vt = sbB.tile([128, 16 * 65], bf16, tag="vt", name="vt", bufs=8)
            nc.vector.memset(vt[:].rearrange("p (g dd) -> p g dd", dd=65)[:, :, 64], 1.0)
            # vt partition order: h_k*8 + t_l (matches score-psum M order);
            # one DMA per t_l so both APs stay 3-dim
            for a in range(8):
                nc.gpsimd.dma_start(
                    vt[:].rearrange("(h t) (grp dd) -> t h grp dd",
                                    t=8, dd=65)[a, :, :, 0:DH],
                    v_dr[:].rearrange("(grp t) (g d) -> t g grp d", t=8, d=DH)[a])
            vts[s] = vt

        def attn_scores(c, Gq, Gk):
            """Scores chunk c (grps 4c..4c+3) -> E [128, 512]."""
            sp = sc_ps.tile([128, 512], fp32, tag="sc", name="sc_ps")
            for g in range(4):
                grp = 4 * c + g
                nc.tensor.matmul(sp[:, 128 * g:128 * (g + 1)],
                                 Gk[:, 128 * grp:128 * (grp + 1)],
                                 Gq[:, 128 * grp:128 * (grp + 1)],
                                 start=True, stop=True)
            E = sbB.tile([128, 512], bf16, tag="E", name="E", bufs=5)
            nc.scalar.activation(E[:], sp[:], Exp)
            nc.vector.tensor_mul(E[:], E[:], mask_sb[:])
            return E

        def attn_ctx(c, E, vts, ctxus):
            """ctx chunk c -> ctxu slice; returns subtile idx when complete."""
            s, cl = divmod(c, 4)
            vt = vts[s]
            cp = cx_ps.tile([128, 260], fp32, tag="cp", name="cx_ps")
            for g in range(4):
                gl = 4 * cl + g   # grp within subtile
                nc.tensor.matmul(cp[:, 65 * g:65 * (g + 1)],
                                 E[:, 128 * g:128 * (g + 1)],
                                 vt[:, 65 * gl:65 * (gl + 1)],
                                 start=True, stop=True)
            nc.scalar.copy(ctxus[s][:, 260 * cl:260 * (cl + 1)], cp[:])

        def subtile_shuffle(ctxu):
            """[(t8,h16), (grp,65)] -> [tok, (h,65)] via DRAM staging."""
            cu_dr = dstage.tile([128, 16 * 65], bf16, tag="cu_dr", name="cu_dr", bufs=2)
            # ctxu partition order: h_q*8 + t_l -> token rows (grp*8 + t_l);
            # one DMA per t_l so both APs stay 3-dim
            for a in range(8):
                nc.gpsimd.dma_start(
                    cu_dr[:].rearrange("(grp t) (h dd) -> t h grp dd",
                                       t=8, dd=65)[a],
                    ctxu[:].rearrange("(h t) (grp dd) -> t h grp dd",
                                      t=8, dd=65)[a])
            ctxf = sbB.tile([128, 16 * 65], bf16, tag="ctxf", name="ctxf", bufs=3)
            nc.sync.dma_start(ctxf[:], cu_dr[:])
            return ctxf

        def normalize(ctxf):
            """DVE normalize: ctxf [tok,(h,65)] -> ctxn [tok, 1024]."""
            rcp = sbB.tile([128, 16], fp32, tag="rcp", name="rcp", bufs=4)
            nc.vector.reciprocal(
                rcp[:], ctxf[:].rearrange("t (h dd) -> t h dd", dd=65)[:, :, 64])
            ctxn = sbB.tile([128, DIM], fp16, tag="ctxn", name="ctxn", bufs=4)
            for h in range(16):
                nc.vector.tensor_scalar_mul(
                    ctxn[:, DH * h:DH * (h + 1)],
                    ctxf[:].rearrange("t (h dd) -> t h dd", dd=65)[:, h, 0:DH],
                    rcp[:, h:h + 1])
            return ctxn

        def p2_transpose(ctxn):
            """ctx transpose for one subtile -> ctxT sbuf (copies on ACT)."""
            ctxT = sbB.tile([128, DIM], fp16, tag="ctxT", name="ctxT", bufs=3)
            for c in range(2):
                tp = tr_ps.tile([128, 512], fp16, tag="trp", name="ctxT_ps")
                for jj in range(4):
                    cc = 4 * c + jj
                    nc.tensor.transpose(tp[:, 128 * jj:128 * (jj + 1)],
                                        ctxn[:, 128 * cc:128 * (cc + 1)], ident[:])
                nc.scalar.copy(ctxT[:, 512 * c:512 * (c + 1)], tp[:])
            return ctxT

        def p2_wo(mm, s, ctxT):
            """Wo projection + store for subtile s of macrotile mm."""
            out_sb = sbB.tile([128, DIM], fp32, tag="out_sb", name="out_sb", bufs=2)
            pss = [proj_ps.tile([128, TM], fp32, tag="proj", name="proj_ps")
                   for _ in range(2)]
            for n in range(2):
                for b in range(8):
                    nc.tensor.matmul(
                        pss[n][:, 0:512], ctxT[:, 128 * b:128 * (b + 1)],
                        w_sb["wo"][:, DIM * b + 512 * n:DIM * b + 512 * (n + 1)],
                        start=(b == 0), stop=(b == 7))
                nc.scalar.copy(out_sb[:, 512 * n:512 * (n + 1)], pss[n][:, 0:512])
            row0 = (4 * mm + s) * 128
            nc.sync.dma_start(out_d[row0:row0 + 128, :], out_sb[:])

        # ---- main pipeline ----
        xts = {0: load_xt(0)}
        Gs, vts_all = {}, {}
        deferred = []
        for m in range(nmacro + 1):
            mm = m - 1
            if m < nmacro:
                if m + 1 < nmacro:
                    xts[m + 1] = load_xt(m + 1)
                Gq = sbA.tile([64, 16 * TM], fp16, tag="Gq", name="Gq", bufs=2)
                Gk = sbA.tile([64, 16 * TM], fp16, tag="Gk", name="Gk", bufs=2)
                Gs[m] = (Gq, Gk)
                vts_all[m] = [None] * 4
            if mm >= 0:
                Gq_p, Gk_p = Gs.pop(mm)
                vts_p = vts_all.pop(mm)
                Es = {}
                ctxus = [sbB.tile([128, 16 * 65], bf16, tag="ctxu",
                                  name="ctxu", bufs=3) for _ in range(4)]
                ctxfs = [None] * 4
                ctxns = [None] * 4

            def emit_attn(sc_list, cx_list):
                for c in sc_list:
                    Es[c] = attn_scores(c, Gq_p, Gk_p)
                for c in cx_list:
                    attn_ctx(c, Es.pop(c), vts_p, ctxus)
                    if c % 4 == 3:
                        ctxfs[c // 4] = subtile_shuffle(ctxus[c // 4])

            # per j-slot attention emission (chunks of macrotile mm):
            #   scores {2j+2, 2j+3} (plus {0,1} at j=0), ctx {2j, 2j+1}
            #   for j>=1 -> subtile s shuffles at j=2s+2 (s=3 at j=7),
            #   normalize(s) two slots later, phase2_pe all in the tail.
            def slot_attn(j):
                if mm < 0:
                    return
                sc = [c for c in ([0, 1] if j == 0 else []) +
                      [2 * j + 2, 2 * j + 3] if c < 16]
                if j == 0:
                    cx = []
                elif j == 1:
                    cx = [0, 1, 2, 3]
                else:
                    cx = [2 * j, 2 * j + 1]
                emit_attn(sc, cx)
                if j in (4, 6, 7):
                    s = {4: 0, 6: 1, 7: 2}[j]
                    ctxns[s] = normalize(ctxfs[s])

            def tail():
                # tp(s+1) overlaps the ACT copies feeding Wo(s);
                # subtile 3 is deferred into the next iteration so its
                # shuffle/DMA latency hides behind j=0 projection work
                cT0 = p2_transpose(ctxns[0])
                cT1 = p2_transpose(ctxns[1])
                p2_wo(mm, 0, cT0)
                cT2 = p2_transpose(ctxns[2])
                p2_wo(mm, 1, cT1)
                p2_wo(mm, 2, cT2)
                return (mm, ctxfs[3])

            def flush_deferred():
                dmm, dctxf = deferred.pop()
                ctxn3 = normalize(dctxf)
                cT3 = p2_transpose(ctxn3)
                p2_wo(dmm, 3, cT3)

            if m < nmacro:
                xt = xts.pop(m)
                for j in range(8):
                    slot_attn(j)
                    proj_qkT(j, xt, Gq, "wq")
                    proj_qkT(j, xt, Gk, "wk")
                    if j == 1 and deferred:
                        flush_deferred()
                    if j % 2 == 0:
                        proj_v(j // 2, xt, vts_all[m])
                if mm >= 0:
                    deferred.append(tail())
            else:
                # drain iteration: attention only
                if deferred:
                    flush_deferred()
                for j in range(8):
                    slot_attn(j)
                deferred.append(tail())
                flush_deferred()

    nc.compile()
    return nc


def _make_mask():
    # within-group col order is h*8 + t_l: same-token iff col%8 == row%8
    m = np.kron(np.ones((16, 16), np.float32), np.eye(8, dtype=np.float32))
    return np.tile(m, (1, 4))  # [128, 512]


def _prep_inputs(x, Wq, Wk, Wv, Wo, nmacro=NMACRO):
    import ml_dtypes
    bf = ml_dtypes.bfloat16
    Tl = TM * nmacro
    w16 = {
        "wq": np.ascontiguousarray(Wq.astype(np.float16)),
        "wk": np.ascontiguousarray(Wk.astype(np.float16)),
        "wv": np.ascontiguousarray(Wv.astype(np.float16)),
        "wo": np.ascontiguousarray(Wo.astype(np.float16)),
    }
    mask = _make_mask().astype(bf)
    ncores = x.shape[0] * x.shape[1] // Tl
    in_maps = []
    for c in range(ncores):
        shard = np.asarray(x).reshape(-1, DIM)[Tl * c:Tl * (c + 1)]
        xT = np.ascontiguousarray(shard.T.astype(np.float16))
        m = {"xT_d": xT, "mask_d": mask}
        for k, v in w16.items():
            m[k + "_d"] = v
        in_maps.append(m)
    return in_maps


def _resolve_names(nc):
    import concourse.mybir as mybir
    in_names, out_name = [], None
    for alloc in nc.m.functions[0].allocations:
        if not isinstance(alloc, mybir.MemoryLocationSet):
            continue
        if alloc.kind == "ExternalInput":
            in_names.append(alloc.memorylocations[0].name)
        elif alloc.kind == "ExternalOutput":
            out_name = alloc.memorylocations[0].name
    return in_names, out_name


def _install_ntff_hook():
    import sys, types
    try:
        from antenv.axon_hooks import get_axon_ntff_profile_hook  # noqa: F401
        return
    except ImportError:
        pass
    try:
        from trn_agent_boot.trn_boot import _ntff_profile_via_ctypes
        hook = _ntff_profile_via_ctypes('/opt/axon/libaxon_pjrt.so')
    except Exception:
        hook = None
    mod = types.ModuleType('antenv.axon_hooks')
    mod._hook = hook
    mod.get_axon_ntff_profile_hook = lambda: mod._hook
    mod.set_axon_ntff_profile_hook = lambda h: setattr(mod, '_hook', h)
    sys.modules['antenv.axon_hooks'] = mod


def kernel(x, Wq, bq, Wk, bk, Wv, bv, Wo, bo, trace=False):
    from concourse.bass_utils import run_bass_kernel_spmd

    if trace:
        _install_ntff_hook()

    if "nc" not in _CACHE:
        _CACHE["nc"] = _build()
    nc = _CACHE["nc"]

    in_names, out_name = _resolve_names(nc)

    def resolve(logical):
        for nm in in_names:
            if nm == logical or nm.startswith(logical + "_") or nm.startswith(logical):
                return nm
        raise KeyError(f"no DRAM tensor matching {logical}: {in_names}")

    raw_maps = _prep_inputs(np.asarray(x), np.asarray(Wq), np.asarray(Wk),
                            np.asarray(Wv), np.asarray(Wo))
    in_maps = [{resolve(k): v for k, v in m.items()} for m in raw_maps]

    res = run_bass_kernel_spmd(nc, in_maps, core_ids=list(range(NCORES)),
                               trace=trace)
    outs = [res.results[c][out_name].reshape(BPC, N, DIM) for c in range(NCORES)]
    full = np.concatenate(outs, axis=0).astype(np.float32)
    if trace:
        kernel.last_exec_time_ns = res.exec_time_ns
    return full


# revision 31
# speedup vs baseline: 1.1395x; 1.1395x over previous
"""Trainium2 Bass kernel v3 for nn_MultiHeadAttention_81655918232272.

Reference semantics:
    q = (x @ Wq).reshape(B, N, H, Dh)   # H=16 heads, Dh=64 (biases zero)
    scores = einsum("bnhd,bngd->bnhg", q, k)   # per-token 16x16 head-mixing
    ctx = softmax(scores, -1) @ v ; out = ctx.reshape(.., 1024) @ Wo

Design v3 (per core: 4096 tokens = 8 macrotiles x 512 tokens):
  - q,k computed TRANSPOSED via swapped-operand matmuls (lhsT=W chunk,
    rhs=xT chunk, N=512): psum [128=(2 heads x 64d), 512 tok]; DVE
    copies build Gq/Gk [64 d, (t512 h16)-interleaved] straight from
    psum -> no PE transposes for q/k.
  - v computed forward per 128-token subtile; ones-column fused vt via
    coarse DMA shuffle through DRAM staging (as v2).
  - scores: per 4-grp chunk, 4 matmuls K=64 -> [128,512] psum; exp on
    ACT; block-diag mask multiply on DVE -> E chunk.
  - ctx: 4 matmuls N=65 -> [128,260] psum -> ctxu; per-subtile DMA
    shuffle back to [tok, (h,65)]; normalize; PE transpose; Wo forward.
  - software pipeline: attention of macrotile m-1 interleaved between
    the projection chunks of macrotile m; ctx staggered 2 chunks behind
    scores so the exp/mask chain never stalls the PE.
"""

import numpy as np

H = 16
DH = 64
DIM = 1024
B, N = 32, 1024
NCORES = 8
BPC = B // NCORES          # batches per core
T = BPC * N                # tokens per core (4096)
NTILE = T // 128           # 128-token tiles per core (32)
TM = 512                   # macrotile tokens
NMACRO = T // TM           # 8

_CACHE = {}


def _build(nmacro=NMACRO, debug=False):
    import concourse.bass as bass  # noqa: F401
    import concourse.mybir as mybir
    import concourse.tile as tile
    from concourse import bacc
    from concourse.masks import make_identity
    from contextlib import ExitStack

    bf16, fp32 = mybir.dt.bfloat16, mybir.dt.float32
    fp16 = mybir.dt.float16
    Exp = mybir.ActivationFunctionType.Exp
    Tl = TM * nmacro

    nc = bacc.Bacc(None, target_bir_lowering=False, debug=debug)

    with tile.TileContext(nc) as tc, ExitStack() as ctx:
        dram = ctx.enter_context(tc.tile_pool(name="dram", bufs=1, space="DRAM"))
        const = ctx.enter_context(tc.tile_pool(name="const", bufs=1))
        sbA = ctx.enter_context(tc.tile_pool(name="sbA", bufs=2))
        sbB = ctx.enter_context(tc.tile_pool(name="sbB", bufs=2))
        dstage = ctx.enter_context(tc.tile_pool(name="dstage", bufs=2, space="DRAM"))
        proj_ps = ctx.enter_context(tc.tile_pool(name="proj_ps", bufs=2, space="PSUM"))
        sc_ps = ctx.enter_context(tc.tile_pool(name="sc_ps", bufs=2, space="PSUM"))
        cx_ps = ctx.enter_context(tc.tile_pool(name="cx_ps", bufs=2, space="PSUM"))
        tr_ps = ctx.enter_context(tc.tile_pool(name="tr_ps", bufs=2, space="PSUM"))

        # ---- DRAM I/O ----
        xT_d = dram.tile([DIM, Tl], fp16, kind="ExternalInput")
        w_d = {}
        for wname in ("wq", "wk", "wv", "wo"):
            w_d[wname] = dram.tile([DIM, DIM], fp16, kind="ExternalInput",
                                   name=f"{wname}_d")
        mask_d = dram.tile([128, 512], bf16, kind="ExternalInput")
        out_d = dram.tile([Tl, DIM], fp32, kind="ExternalOutput")

        # ---- resident SBUF ----
        w_sb = {}
        for wname in ("wq", "wk", "wv", "wo"):
            wt = const.tile([128, 8 * DIM], fp16, tag=f"w_{wname}", name=f"w_{wname}_sb")
            for kt in range(8):
                nc.sync.dma_start(wt[:, DIM * kt:DIM * (kt + 1)],
                                  w_d[wname][128 * kt:128 * (kt + 1), :])
            w_sb[wname] = wt
        mask_sb = const.tile([128, 512], bf16)
        nc.sync.dma_start(mask_sb[:], mask_d[:])
        ident = const.tile([128, 128], fp16)
        make_identity(nc, ident[:])

        def load_xt(m):
            t0 = TM * m
            xt = sbA.tile([128, 8, TM], fp16, tag="xt", name="xt", bufs=2)
            nc.sync.dma_start(
                xt[:],
                xT_d[:, t0:t0 + TM].rearrange("(kt f) t -> f kt t", f=128))
            return xt

        def proj_qkT(j, xt, G, wname):
            """Transposed projection chunk j: psum [(2h,64d), 512 tok] ->
            DVE-interleave into G [64, (t 512, h 16)]."""
            ps = proj_ps.tile([128, TM], fp32, tag="proj", name="proj_ps")
            for kt in range(8):
                nc.tensor.matmul(
                    ps[:],
                    w_sb[wname][:, DIM * kt + 128 * j:DIM * kt + 128 * (j + 1)],
                    xt[:, kt, :],
                    start=(kt == 0), stop=(kt == 7))
            # dense ACT cast psum->fp16, then DVE interleave into G.
            # G col order: grp*128 + h*8 + t_l  (h-major within group)
            qs = sbA.tile([128, TM], fp16, tag="qks", name="qT_sb", bufs=2)
            nc.scalar.copy(qs[:], ps[:])
            Gv = G[:].rearrange("d (grp hh t) -> d grp hh t", hh=H, t=8)
            for p in range(2):
                nc.vector.tensor_copy(
                    Gv[:, :, 2 * j + p, :],
                    qs[64 * p:64 * (p + 1), :].rearrange("d (grp t) -> d grp t", t=8))

        def proj_v(s, xt, vts):
            """Forward V projection for subtile s + vt shuffle."""
            v16 = sbA.tile([128, DIM], bf16, tag="v16", name="v16", bufs=2)
            pss = [proj_ps.tile([128, TM], fp32, tag="proj", name="proj_ps")
                   for _ in range(2)]
            for n in range(2):
                for kt in range(8):
                    nc.tensor.matmul(
                        pss[n][:, 0:512],
                        xt[:, kt, 128 * s:128 * (s + 1)],
                        w_sb["wv"][:, DIM * kt + 512 * n:DIM * kt + 512 * (n + 1)],
                        start=(kt == 0), stop=(kt == 7))
                nc.scalar.copy(v16[:, 512 * n:512 * (n + 1)], pss[n][:, 0:512])

            v_dr = dstage.tile([128, DIM], bf16, tag="v_dr", name="v_dr", bufs=2)
            nc.sync.dma_start(v_dr[:], v16[:])
            vt = sbB.tile([128, 16 * 65], bf16, tag="vt", name="vt", bufs=8)
            nc.vector.memset(vt[:].rearrange("p (g dd) -> p g dd", dd=65)[:, :, 64], 1.0)
            # vt partition order: h_k*8 + t_l (matches score-psum M order);
            # one DMA per t_l so both APs stay 3-dim
            for a in range(8):
                nc.gpsimd.dma_start(
                    vt[:].rearrange("(h t) (grp dd) -> t h grp dd",
                                    t=8, dd=65)[a, :, :, 0:DH],
                    v_dr[:].rearrange("(grp t) (g d) -> t g grp d", t=8, d=DH)[a])
            vts[s] = vt

        def attn_scores(c, Gq, Gk):
            """Scores chunk c (grps 4c..4c+3) -> E [128, 512]."""
            sp = sc_ps.tile([128, 512], fp32, tag="sc", name="sc_ps")
            for g in range(4):
                grp = 4 * c + g
                nc.tensor.matmul(sp[:, 128 * g:128 * (g + 1)],
                                 Gk[:, 128 * grp:128 * (grp + 1)],
                                 Gq[:, 128 * grp:128 * (grp + 1)],
                                 start=True, stop=True)
            E = sbB.tile([128, 512], bf16, tag="E", name="E", bufs=5)
            nc.scalar.activation(E[:], sp[:], Exp)
            nc.vector.tensor_mul(E[:], E[:], mask_sb[:])
            return E

        def attn_ctx(c, E, vts, ctxus):
            """ctx chunk c -> ctxu slice; returns subtile idx when complete."""
            s, cl = divmod(c, 4)
            vt = vts[s]
            cp = cx_ps.tile([128, 260], fp32, tag="cp", name="cx_ps")
            for g in range(4):
                gl = 4 * cl + g   # grp within subtile
                nc.tensor.matmul(cp[:, 65 * g:65 * (g + 1)],
                                 E[:, 128 * g:128 * (g + 1)],
                                 vt[:, 65 * gl:65 * (gl + 1)],
                                 start=True, stop=True)
            nc.scalar.copy(ctxus[s][:, 260 * cl:260 * (cl + 1)], cp[:])

        def subtile_shuffle(ctxu):
            """[(t8,h16), (grp,65)] -> [tok, (h,65)] via DRAM staging."""
            cu_dr = dstage.tile([128, 16 * 65], bf16, tag="cu_dr", name="cu_dr", bufs=2)
            # ctxu partition order: h_q*8 + t_l -> token rows (grp*8 + t_l);
            # one DMA per t_l so both APs stay 3-dim
            for a in range(8):
                nc.gpsimd.dma_start(
                    cu_dr[:].rearrange("(grp t) (h dd) -> t h grp dd",
                                       t=8, dd=65)[a],
                    ctxu[:].rearrange("(h t) (grp dd) -> t h grp dd",
                                      t=8, dd=65)[a])
            ctxf = sbB.tile([128, 16 * 65], bf16, tag="ctxf", name="ctxf", bufs=3)
            nc.sync.dma_start(ctxf[:], cu_dr[:])
            return ctxf

        def normalize(ctxf):
            """DVE normalize: ctxf [tok,(h,65)] -> ctxn [tok, 1024]."""
            rcp = sbB.tile([128, 16], fp32, tag="rcp", name="rcp", bufs=4)
            nc.vector.reciprocal(
                rcp[:], ctxf[:].rearrange("t (h dd) -> t h dd", dd=65)[:, :, 64])
            ctxn = sbB.tile([128, DIM], fp16, tag="ctxn", name="ctxn", bufs=4)
            for h in range(16):
                nc.vector.tensor_scalar_mul(
                    ctxn[:, DH * h:DH * (h + 1)],
                    ctxf[:].rearrange("t (h dd) -> t h dd", dd=65)[:, h, 0:DH],
                    rcp[:, h:h + 1])
            return ctxn

        def p2_transpose(ctxn):
            """ctx transpose for one subtile -> ctxT sbuf (copies on ACT)."""
            ctxT = sbB.tile([128, DIM], fp16, tag="ctxT", name="ctxT", bufs=3)
            for c in range(2):
                tp = tr_ps.tile([128, 512], fp16, tag="trp", name="ctxT_ps")
                for jj in range(4):
                    cc = 4 * c + jj
                    nc.tensor.transpose(tp[:, 128 * jj:128 * (jj + 1)],
                                        ctxn[:, 128 * cc:128 * (cc + 1)], ident[:])
                nc.scalar.copy(ctxT[:, 512 * c:512 * (c + 1)], tp[:])
            return ctxT

        def p2_wo(mm, s, ctxT):
            """Wo projection + store for subtile s of macrotile mm."""
            out_sb = sbB.tile([128, DIM], fp32, tag="out_sb", name="out_sb", bufs=2)
            pss = [proj_ps.tile([128, TM], fp32, tag="proj", name="proj_ps")
                   for _ in range(2)]
            for n in range(2):
                for b in range(8):
                    nc.tensor.matmul(
                        pss[n][:, 0:512], ctxT[:, 128 * b:128 * (b + 1)],
                        w_sb["wo"][:, DIM * b + 512 * n:DIM * b + 512 * (n + 1)],
                        start=(b == 0), stop=(b == 7))
                nc.scalar.copy(out_sb[:, 512 * n:512 * (n + 1)], pss[n][:, 0:512])
            row0 = (4 * mm + s) * 128
            nc.sync.dma_start(out_d[row0:row0 + 128, :], out_sb[:])

        # ---- main pipeline ----
        xts = {0: load_xt(0)}
        Gs, vts_all = {}, {}
        deferred = []
        for m in range(nmacro + 1):
            mm = m - 1
            if m < nmacro:
                if m + 1 < nmacro:
                    xts[m + 1] = load_xt(m + 1)
                Gq = sbA.tile([64, 16 * TM], fp16, tag="Gq", name="Gq", bufs=2)
                Gk = sbA.tile([64, 16 * TM], fp16, tag="Gk", name="Gk", bufs=2)
                Gs[m] = (Gq, Gk)
                vts_all[m] = [None] * 4
            if mm >= 0:
                Gq_p, Gk_p = Gs.pop(mm)
                vts_p = vts_all.pop(mm)
                Es = {}
                ctxus = [sbB.tile([128, 16 * 65], bf16, tag="ctxu",
                                  name="ctxu", bufs=3) for _ in range(4)]
                ctxfs = [None] * 4
                ctxns = [None] * 4

            def emit_attn(sc_list, cx_list):
                for c in sc_list:
                    Es[c] = attn_scores(c, Gq_p, Gk_p)
                for c in cx_list:
                    attn_ctx(c, Es.pop(c), vts_p, ctxus)
                    if c % 4 == 3:
                        ctxfs[c // 4] = subtile_shuffle(ctxus[c // 4])

            # per j-slot attention emission (chunks of macrotile mm):
            #   scores {2j+2, 2j+3} (plus {0,1} at j=0), ctx {2j, 2j+1}
            #   for j>=1 -> subtile s shuffles at j=2s+2 (s=3 at j=7),
            #   normalize(s) two slots later, phase2_pe all in the tail.
            def slot_attn(j):
                if mm < 0:
                    return
                sc = [c for c in ([0, 1] if j == 0 else []) +
                      [2 * j + 2, 2 * j + 3] if c < 16]
                if j == 0:
                    cx = []
                elif j == 1:
                    cx = [0, 1, 2, 3]
                else:
                    cx = [2 * j, 2 * j + 1]
                emit_attn(sc, cx)
                if j in (4, 6, 7):
                    s = {4: 0, 6: 1, 7: 2}[j]
                    ctxns[s] = normalize(ctxfs[s])

            def tail():
                # tp(s+1) overlaps the ACT copies feeding Wo(s)
                cT0 = p2_transpose(ctxns[0])
                cT1 = p2_transpose(ctxns[1])
                p2_wo(mm, 0, cT0)
                ctxns[3] = normalize(ctxfs[3])
                cT2 = p2_transpose(ctxns[2])
                p2_wo(mm, 1, cT1)
                cT3 = p2_transpose(ctxns[3])
                p2_wo(mm, 2, cT2)
                p2_wo(mm, 3, cT3)

            if m < nmacro:
                xt = xts.pop(m)
                for j in range(8):
                    slot_attn(j)
                    proj_qkT(j, xt, Gq, "wq")
                    proj_qkT(j, xt, Gk, "wk")
                    if j % 2 == 0:
                        proj_v(j // 2, xt, vts_all[m])
                if mm >= 0:
                    tail()
            else:
                # drain iteration: attention only
                for j in range(8):
                    slot_attn(j)
                tail()

    nc.compile()
    return nc


def _make_mask():
    # within-group col order is h*8 + t_l: same-token iff col%8 == row%8
    m = np.kron(np.ones((16, 16), np.float32), np.eye(8, dtype=np.float32))
    return np.tile(m, (1, 4))  # [128, 512]


def _prep_inputs(x, Wq, Wk, Wv, Wo, nmacro=NMACRO):
    import ml_dtypes
    bf = ml_dtypes.bfloat16
    Tl = TM * nmacro
    w16 = {
        "wq": np.ascontiguousarray(Wq.astype(np.float16)),
        "wk": np.ascontiguousarray(Wk.astype(np.float16)),
        "wv": np.ascontiguousarray(Wv.astype(np.float16)),
        "wo": np.ascontiguousarray(Wo.astype(np.float16)),
    }
    mask = _make_mask().astype(bf)
    ncores = x.shape[0] * x.shape[1] // Tl
    in_maps = []
    for c in range(ncores):
        shard = np.asarray(x).reshape(-1, DIM)[Tl * c:Tl * (c + 1)]
        xT = np.ascontiguousarray(shard.T.astype(np.float16))
        m = {"xT_d": xT, "mask_d": mask}
        for k, v in w16.items():
            m[k + "_d"] = v
        in_maps.append(m)
    return in_maps


def _resolve_names(nc):
    import concourse.mybir as mybir
    in_names, out_name = [], None
    for alloc in nc.m.functions[0].allocations:
        if not isinstance(alloc, mybir.MemoryLocationSet):
            continue
        if alloc.kind == "ExternalInput":
            in_names.append(alloc.memorylocations[0].name)
        elif alloc.kind == "ExternalOutput":
            out_name = alloc.memorylocations[0].name
    return in_names, out_name


def _install_ntff_hook():
    import sys, types
    try:
        from antenv.axon_hooks import get_axon_ntff_profile_hook  # noqa: F401
        return
    except ImportError:
        pass
    try:
        from trn_agent_boot.trn_boot import _ntff_profile_via_ctypes
        hook = _ntff_profile_via_ctypes('/opt/axon/libaxon_pjrt.so')
    except Exception:
        hook = None
    mod = types.ModuleType('antenv.axon_hooks')
    mod._hook = hook
    mod.get_axon_ntff_profile_hook = lambda: mod._hook
    mod.set_axon_ntff_profile_hook = lambda h: setattr(mod, '_hook', h)
    sys.modules['antenv.axon_hooks'] = mod


def kernel(x, Wq, bq, Wk, bk, Wv, bv, Wo, bo, trace=False):
    from concourse.bass_utils import run_bass_kernel_spmd

    if trace:
        _install_ntff_hook()

    if "nc" not in _CACHE:
        _CACHE["nc"] = _build()
    nc = _CACHE["nc"]

    in_names, out_name = _resolve_names(nc)

    def resolve(logical):
        for nm in in_names:
            if nm == logical or nm.startswith(logical + "_") or nm.startswith(logical):
                return nm
        raise KeyError(f"no DRAM tensor matching {logical}: {in_names}")

    raw_maps = _prep_inputs(np.asarray(x), np.asarray(Wq), np.asarray(Wk),
                            np.asarray(Wv), np.asarray(Wo))
    in_maps = [{resolve(k): v for k, v in m.items()} for m in raw_maps]

    res = run_bass_kernel_spmd(nc, in_maps, core_ids=list(range(NCORES)),
                               trace=trace)
    outs = [res.results[c][out_name].reshape(BPC, N, DIM) for c in range(NCORES)]
    full = np.concatenate(outs, axis=0).astype(np.float32)
    if trace:
        kernel.last_exec_time_ns = res.exec_time_ns
    return full
